# revision 10
# baseline (speedup 1.0000x reference)
"""DistanceLoss (EDT + weighted softmax loss) on 8 Trainium2 NeuronCores.

Sharding: data-parallel over batch. Each of the 8 cores processes 2 of the 16
batch samples (all 5 classes), computing for every (b, c) slice:
  - exact 1D row... (column-direction) distances via two prefix scans,
  - exact 2D squared EDT via a banded min-plus over column offsets |delta|<=R,
  - per-class partial sums S1 = sum(probs*d), S2 = sum_present(probs),
    and max(d^2) per slice (also used to verify the band radius was enough).
The host combines: loss = sum_{b,c} w_c/sum(w) * (S1 - sqrt(maxd2)*S2) / N.

The band radius R is exact whenever the true max EDT distance <= R; this is
verified on the gathered maxd2 values, with a pure-numpy exact fallback
otherwise (never taken for the target input distribution).
"""

import numpy as np

B, C, H, W = 16, 5, 256, 256
NCORES = 8
BPC = B // NCORES  # batches per core
R = 6              # min-plus band radius (exact iff max EDT distance <= R)
BIG = 512.0        # reference's 1D-distance clamp (H + W)
P = 128

_CACHE = {}


def _build_nc(legalize=True):
    import concourse.bass as bass
    import concourse.mybir as mybir
    import concourse.tile as tile
    from concourse import masks

    f32 = mybir.dt.float32
    i32 = mybir.dt.int32
    Alu = mybir.AluOpType
    Act = mybir.ActivationFunctionType

    nc = bass.Bass()
    pred_d = nc.dram_tensor("predictions", [BPC, C, H, W], f32, kind="ExternalInput")
    tgt_d = nc.dram_tensor("targets", [BPC, H, W], i32, kind="ExternalInput")
    # stats columns: [0:10] S1 (b*5+c), [10:20] S2, [20:30] maxd2, [30:32] pad
    out_d = nc.dram_tensor("out_stats", [P, 32], f32, kind="ExternalOutput")

    with tile.TileContext(nc) as tc:
        with (
            tc.tile_pool(name="const", bufs=1) as cpool,
            tc.tile_pool(name="work", bufs=2) as pool,
            tc.tile_pool(name="psum", bufs=4, space="PSUM") as psum,
        ):
            # Build a 128x128 identity using only DVE (so the later ACT copy
            # makes every PE-transpose input ACT-produced -> each Matmult
            # needs at most ONE sync wait; walrus rejects multi-wait LWs).
            ones = cpool.tile([P, 2 * (H + 1)], f32)
            nc.vector.memset(ones[:], 1.0)
            rowv = cpool.tile([P, P], f32)  # rowv[p, f] = f + 1
            nc.vector.tensor_tensor_scan(
                rowv[:], ones[:, :P], ones[:, :P], 0.0, Alu.add, Alu.mult)
            colv = cpool.tile([P, 1], f32)  # colv[p] = p + 1
            colm = cpool.tile([P, 32], f32)
            for a in range(4):
                nc.vector.transpose(
                    colm[a * 32:(a + 1) * 32, :],
                    rowv[a * 32:(a + 1) * 32, a * 32:(a + 1) * 32])
            nc.vector.tensor_copy(colv[:], colm[:, :1])
            identD = cpool.tile([P, P], f32)
            nc.vector.tensor_scalar(
                identD[:], rowv[:], colv[:], None, Alu.is_equal)
            ident = cpool.tile([P, P], f32)
            nc.scalar.copy(ident[:], identD[:])

            stats = cpool.tile([P, 32], f32)
            nc.vector.memset(stats[:], 0.0)

            for b in range(BPC):
                # ---- load targets, cast, transpose to [w, h] layout ----
                t_i32 = pool.tile([P, 2, W], i32)
                nc.sync.dma_start(
                    t_i32[:], tgt_d[b].rearrange("(n p) w -> p n w", p=P))
                t_nat = pool.tile([P, 2, W], f32)  # [h(part), hb, w]
                nc.vector.tensor_copy(t_nat[:], t_i32[:])
                t_natA = pool.tile([P, 2, W], f32)  # ACT-produced copy for PE
                nc.scalar.copy(t_natA[:], t_nat[:])

                t_T = pool.tile([P, 2, H], f32)  # [w(part), wb, h]
                for wb in range(2):
                    pt = psum.tile([P, H], f32, tag="pt")
                    for hb in range(2):
                        nc.tensor.transpose(
                            pt[:, hb * P:(hb + 1) * P],
                            t_natA[:, hb, wb * P:(wb + 1) * P], ident[:])
                    nc.scalar.copy(t_T[:, wb, :], pt[:])

                # ---- softmax pieces: e = exp(pred), q = 1/sum_c e ----
                pred = pool.tile([P, 2, C, W], f32)  # [h(part), hb, c, w]
                pred_v = pred_d[b].rearrange("c (n p) w -> p n c w", p=P)
                for hb in range(2):
                    nc.sync.dma_start(pred[:, hb], pred_v[:, hb])
                e_all = pool.tile([P, 2, C, W], f32)
                nc.scalar.activation(e_all[:], pred[:], Act.Exp)
                s = pool.tile([P, 2, W], f32)
                nc.vector.tensor_reduce(
                    s[:], e_all[:].transpose([0, 1, 3, 2]),
                    mybir.AxisListType.X, Alu.add)
                q = pool.tile([P, 2, W], f32)
                nc.vector.reciprocal(q[:], s[:])
                eq = pool.tile([P, 2, C, W], f32)  # probs = e * (1/s)
                nc.vector.tensor_tensor(
                    eq[:], e_all[:],
                    q[:].unsqueeze(2).broadcast_to([P, 2, C, W]), Alu.mult)

                # ---- per class: 1D scans (column direction) -> g^2, transposed ----
                G2 = pool.tile([P, 2, C, W], f32)  # [h(part), hb, c, w]
                HP = H + 1  # padded scan length; pad breaks the wb0->wb1 chain
                for c in range(C):
                    notpT = pool.tile([P, 2, HP], f32, tag="notpT")
                    nc.vector.tensor_scalar(
                        notpT[:, :, :H], t_T[:], float(c), None, Alu.not_equal)
                    nc.vector.memset(notpT[:, :, H:], 1.0e6)
                    fwd = pool.tile([P, 2, HP], f32, tag="fwd")
                    bwd = pool.tile([P, 2, HP], f32, tag="bwd")
                    nfl = notpT[:].rearrange("p a h -> p (a h)")
                    nc.vector.tensor_tensor_scan(
                        fwd[:].rearrange("p a h -> p (a h)"),
                        ones[:, :2 * HP], nfl, BIG, Alu.add, Alu.mult)
                    nc.vector.tensor_tensor_scan(
                        bwd[:].rearrange("p a h -> p (a h)")[:, ::-1],
                        ones[:, :2 * HP], nfl[:, ::-1], BIG, Alu.add, Alu.mult)
                    g = pool.tile([P, 2, H], f32, tag="g")
                    nc.vector.tensor_tensor(
                        g[:], fwd[:, :, :H], bwd[:, :, :H], Alu.min)
                    # NOTE: the reference's min(g, BIG) clamp is dropped: it
                    # only changes results when max(d) > R, which the band
                    # check catches (-> exact host fallback).
                    # Square on ACT (so the PE transpose input is ACT-made),
                    # transpose g^2 into natural [h, w] layout.
                    g2T = pool.tile([P, 2, H], f32, tag="g2T")
                    nc.scalar.activation(g2T[:], g[:], Act.Square)
                    for hb in range(2):
                        pg = psum.tile([P, W], f32, tag="pg")
                        for wb in range(2):
                            nc.tensor.transpose(
                                pg[:, wb * P:(wb + 1) * P],
                                g2T[:, wb, hb * P:(hb + 1) * P], ident[:])
                        nc.scalar.copy(G2[:, hb, c, :], pg[:])

                # ---- banded min-plus along w: d2 = min_d (d^2 + G2 shifted) ----
                d2 = pool.tile([P, 2, C, W], f32)
                nc.vector.tensor_copy(d2[:], G2[:])
                for dlt in range(1, R + 1):
                    cc = float(dlt * dlt)
                    nc.vector.scalar_tensor_tensor(
                        d2[:, :, :, dlt:], G2[:, :, :, :W - dlt], cc,
                        d2[:, :, :, dlt:], Alu.add, Alu.min)
                    nc.vector.scalar_tensor_tensor(
                        d2[:, :, :, :W - dlt], G2[:, :, :, dlt:], cc,
                        d2[:, :, :, :W - dlt], Alu.add, Alu.min)

                # ---- per-slice max(d2) + d = sqrt(d2) ----
                for c in range(C):
                    col = 20 + b * C + c
                    nc.vector.tensor_reduce(
                        stats[:, col:col + 1], d2[:, :, c, :],
                        mybir.AxisListType.XY, Alu.max)
                d_all = pool.tile([P, 2, C, W], f32)
                nc.scalar.activation(d_all[:], d2[:], Act.Sqrt)

                # ---- per-class partial sums ----
                # S1 = sum(d * probs); S2 = sum(probs where target == c).
                junk = pool.tile([P, 2, W], f32, tag="junk")
                for c in range(C):
                    c1 = b * C + c
                    nc.vector.scalar_tensor_tensor(
                        junk[:], d_all[:, :, c, :], 0.0, eq[:, :, c, :],
                        Alu.add, Alu.mult,
                        accum_out=stats[:, c1:c1 + 1])
                    nc.vector.scalar_tensor_tensor(
                        junk[:], t_nat[:], float(c), eq[:, :, c, :],
                        Alu.is_equal, Alu.mult,
                        accum_out=stats[:, 10 + c1:11 + c1])

            nc.sync.dma_start(out_d[:], stats[:])

    if not legalize:  # CoreSim path: its race detector rejects raw NoOps
        return nc

    # walrus codegen in this toolchain allows only ONE sync wait per
    # instruction; split extras onto same-engine NoOps inserted right before
    # (engine streams are in-order, so this is semantically identical). It
    # also rejects the EVENT_SEMAPHORE_RANGE_CLEAR encoding; replace it with
    # per-semaphore `sem-wr-imm 0` updates on NoOps.
    rc_op = nc.isa.Opcode.NEURON_ISA_TPB_OPCODE_EVENT_SEMAPHORE_RANGE_CLEAR.value
    for f in nc.m.functions:
        for blk in f.blocks:
            newlist = []
            for inst in blk.instructions:
                si = inst.sync_info
                if si is not None and si.on_wait and len(si.on_wait) > 1:
                    for w in si.on_wait[:-1]:
                        newlist.append(mybir.InstNoOp(
                            name=nc.get_next_instruction_name(),
                            engine=inst.engine,
                            bass_nofuse=True,
                            sync_info=mybir.SyncInfo(on_wait=[w], on_update=[]),
                        ))
                    si.on_wait = [si.on_wait[-1]]
                if (isinstance(inst, mybir.InstISA)
                        and inst.isa_opcode == rc_op):
                    struct = inst.ant_dict
                    for semid in range(struct["range_first"],
                                       struct["range_last"] + 1):
                        newlist.append(mybir.InstNoOp(
                            name=nc.get_next_instruction_name(),
                            engine=inst.engine,
                            bass_nofuse=True,
                            sync_info=mybir.SyncInfo(
                                on_wait=list(si.on_wait) if (
                                    si and semid == struct["range_first"]
                                ) else [],
                                on_update=[mybir.SyncUpdate(
                                    sync_type="semaphore", id=semid,
                                    update_mode="sem-wr-imm",
                                    update_value=0)],
                            ),
                        ))
                    continue
                newlist.append(inst)
            blk.instructions[:] = newlist
    return nc


def _numpy_fallback(predictions, weight, targets):
    """Exact reimplementation of the reference in numpy (float32 math)."""
    predictions = np.asarray(predictions, np.float32)
    targets = np.asarray(targets)
    weight = np.asarray(weight, np.float32)
    Bf, Cf, Hf, Wf = predictions.shape
    big = np.float32(Hf + Wf)
    total = np.float64(0.0)
    wn = (weight / weight.sum()).astype(np.float32)
    for b in range(Bf):
        pm = predictions[b] - predictions[b].max(axis=0, keepdims=True)
        ex = np.exp(pm, dtype=np.float32)
        probs = ex / ex.sum(axis=0, keepdims=True)
        for c in range(Cf):
            p = (targets[b] == c)
            notp = ~p
            # 1D row distances with BIG init/clamp (scan along axis 1)
            fwd = np.zeros((Hf, Wf), np.float32)
            st = np.full((Hf,), big, np.float32)
            for t in range(Wf):
                st = np.where(notp[:, t], st + 1.0, 0.0)
                fwd[:, t] = st
            bwd = np.zeros((Hf, Wf), np.float32)
            st = np.full((Hf,), big, np.float32)
            for t in range(Wf - 1, -1, -1):
                st = np.where(notp[:, t], st + 1.0, 0.0)
                bwd[:, t] = st
            g = np.minimum(np.minimum(fwd, bwd), big)
            i = np.arange(Hf, dtype=np.float32)
            A = (i[:, None] - i[None, :]) ** 2
            d2 = (A[:, :, None] + (g * g)[None, :, :]).min(axis=1)
            d = np.sqrt(d2)
            dist = np.where(p, np.float32(-1.0) * d.max(), d)
            total += np.float64((probs[c] * dist).sum(dtype=np.float64)) * wn[c]
    return np.float32(total / (Bf * Cf * Hf * Wf))


def kernel(predictions, weight, targets):
    predictions = np.ascontiguousarray(np.asarray(predictions, np.float32))
    targets = np.ascontiguousarray(np.asarray(targets, np.int32))
    weight = np.asarray(weight, np.float32)

    safe_inputs = (
        np.all(np.isfinite(weight)) and np.all(weight > 0)
        and np.all(np.isfinite(predictions))
        and float(np.abs(predictions).max()) < 80.0
    )
    if not safe_inputs:
        return _numpy_fallback(predictions, weight, targets)

    from concourse.bass_utils import run_bass_kernel_spmd

    if "nc" not in _CACHE:
        _CACHE["nc"] = _build_nc()
    nc = _CACHE["nc"]

    in_maps = [
        {
            "predictions": predictions[i * BPC:(i + 1) * BPC],
            "targets": targets[i * BPC:(i + 1) * BPC],
        }
        for i in range(NCORES)
    ]
    res = run_bass_kernel_spmd(nc, in_maps, core_ids=list(range(NCORES)))
    stats = np.stack([r["out_stats"] for r in res.results])  # [8, 128, 32]

    S1 = stats[:, :, 0:10].sum(axis=1, dtype=np.float64).reshape(NCORES, BPC, C)
    S2 = stats[:, :, 10:20].sum(axis=1, dtype=np.float64).reshape(NCORES, BPC, C)
    maxd2 = stats[:, :, 20:30].max(axis=1).reshape(NCORES, BPC, C)

    if maxd2.max() > float(R * R):
        return _numpy_fallback(predictions, weight, targets)

    M = np.sqrt(maxd2.astype(np.float32)).astype(np.float64)
    wn = (weight / weight.sum()).astype(np.float64)
    loss = ((S1 - M * S2) * wn[None, None, :]).sum() / float(B * C * H * W)
    return np.float32(loss)
